# revision 13
# baseline (speedup 1.0000x reference)
"""Trainium2 Bass kernel for nn_Attention_38405597560936.

GroupNorm -> qkv 1x1 conv -> 8-head self-attention over 48x48 tokens -> proj
1x1 conv -> residual.  Sharded over 8 NeuronCores: data-parallel over batch
(2) x tensor-parallel over head pairs (4).  Each core computes GN stats for
its batch, q/k/v for its 2 heads, the attention, and a partial proj output
(contracting only its 128 a-channels); the host sums the 4 partials per
batch and adds proj bias + v-bias contribution + residual.

v2 design (all-bf16 matmul pipeline):
  - GroupNorm is folded into the qkv weights on device: per-channel
    (sc, bi) from the group stats scale the weight columns (w_eff = w*sc)
    and contribute a runtime bias (w @ bi), so the big normalized-x tensor
    never exists and qkv matmuls read raw x (bf16).
  - Attention runs in 5 units over the query axis: 4 single-head 1024-wide
    units + one 256-wide both-heads tail.  Per (unit, token-tile): one QK
    matmul (bf16, 1024-wide moving), one Exp activation, one AV matmul.
  - Softmax denominator via a ones-column appended to v^T (U[64] = den).
  - Layout: q_sb/k_sb/v_sb [128, 2304], partitions 0:64 head A, 64:128 B.
"""
import numpy as np
import ml_dtypes
from contextlib import ExitStack, nullcontext

import concourse.bass as bass
import concourse.tile as tile
from concourse import bacc, mybir
from concourse import bass_utils

F32 = mybir.dt.float32
BF16 = mybir.dt.bfloat16
MMDT = BF16
AF = mybir.ActivationFunctionType
ALU = mybir.AluOpType

B, C, H, W = 2, 512, 48, 48
N = H * W                      # 2304 tokens
HEADS, D = 8, 64
GROUPS = 32                    # 16 channels per group
EPS = 1e-5
SCALE = 1.0 / 8.0              # 1/sqrt(64)
NCORES = 8
CT = C // 128                  # 4 channel tiles
NT = N // 128                  # 18 token tiles
# query chunks (both heads per chunk; head A at cols 0:cw, B at 512:512+cw)
CHUNKS = [(0, 512), (512, 512), (1024, 512), (1536, 512), (2048, 256)]

_CACHE: dict = {}

PRO = 8          # QK/exp software-prologue depth per unit


def _build(phases="abc", repeat=None, warm=True, pro=None, unroll=1):
    assert repeat is None or repeat % unroll == 0
    nc = bacc.Bacc("TRN2", debug=False, num_devices=NCORES)

    # x, c-tile major: [128, 4*2304] bf16
    xbf = nc.dram_tensor("xbf", [128, CT * N], BF16, kind="ExternalInput").ap()
    # packed consts: fpk = [wq(512) | wk(512) | wv(512) | ident(128) | ones(37)]
    fpk = nc.dram_tensor("fpk", [128, 1701], MMDT, kind="ExternalInput").ap()
    wp = nc.dram_tensor("wp", [64, 1024], MMDT, kind="ExternalInput").ap()
    # cpk = [ind(128) | gnsc(4) | gnbi(4) | bq(1) | bk(1)]
    cpk = nc.dram_tensor("cpk", [128, 138], F32, kind="ExternalInput").ap()
    indT = nc.dram_tensor("indT", [32, 512], F32, kind="ExternalInput").ap()

    out = nc.dram_tensor("out", [C, N], F32, kind="ExternalOutput").ap()

    PRO = pro if pro is not None else globals()["PRO"]
    with tile.TileContext(nc) as tc, ExitStack() as ctx:
        pers = ctx.enter_context(tc.tile_pool(name="pers", bufs=1))
        # PSUM: qk (2 slots x 2 banks) | u (2 banks) | pp 1 | tr 1  = 8 banks
        ps = ctx.enter_context(tc.tile_pool(name="ps", bufs=1, space="PSUM"))
        work = ctx.enter_context(tc.tile_pool(name="work", bufs=1))
        xp = ctx.enter_context(tc.tile_pool(name="xp", bufs=2))
        att = ctx.enter_context(tc.tile_pool(name="att", bufs=3))
        nrm = ctx.enter_context(tc.tile_pool(name="nrm", bufs=1))
        wef = ctx.enter_context(tc.tile_pool(name="wef", bufs=2))

        fpk_sb = pers.tile([128, 1701], MMDT)
        nc.gpsimd.dma_start(fpk_sb, fpk)
        wp_sb = pers.tile([64, 1024], MMDT)
        nc.gpsimd.dma_start(wp_sb, wp)
        cpk_sb = pers.tile([128, 138], F32)
        nc.gpsimd.dma_start(cpk_sb, cpk)
        indT_sb = pers.tile([32, 512], F32)
        nc.gpsimd.dma_start(indT_sb, indT)
        wq_sb = fpk_sb[:, 0:512]
        wk_sb = fpk_sb[:, 512:1024]
        wv_sb = fpk_sb[:, 1024:1536]
        ident = fpk_sb[:, 1536:1664]
        wpa_sb = wp_sb[:, 0:512]
        wpb_sb = wp_sb[:, 512:1024]
        ind_sb = cpk_sb[:, 0:128]
        gnsc_sb = cpk_sb[:, 128:132]
        gnbi_sb = cpk_sb[:, 132:136]
        bq_sb = cpk_sb[:, 136:137]
        bk_sb = cpk_sb[:, 137:138]

        # double-buffered across unrolled body copies so copy i+1's qkv prep
        # can overlap copy i's attention tail
        q_pair = [pers.tile([128, N], MMDT, name=f"qb{i}") for i in range(2)]
        k_pair = [pers.tile([128, N], MMDT, name=f"kb{i}") for i in range(2)]
        v_pair = [pers.tile([128, N], MMDT, name=f"vb{i}") for i in range(2)]
        vt_pair = [pers.tile([128, NT * 130], MMDT, name=f"vtb{i}")
                   for i in range(2)]              # [vA|1|vB|1] per token tile
        for i in range(2):
            vt3i = vt_pair[i].rearrange("p (t c) -> p t c", c=130)
            # constant ones columns of vt (positions 64 and 129 of each tile)
            nc.sync.dma_start(vt3i[:, :, 64:65], fpk[:, 1665:1683])
            nc.sync.dma_start(vt3i[:, :, 129:130], fpk[:, 1683:1701])

        eps_t = pers.tile([32, 1], F32)
        nc.vector.memset(eps_t, EPS)

        if warm:
            for _ in range(16):
                warm_t = ps.tile([128, 1024], F32, tag="qk", bufs=2, name="warm")
                nc.tensor.matmul(warm_t[:, 0:512], wq_sb[:, 0:128],
                                 fpk_sb[:, 0:512], start=True, stop=True)

        def body(bi):
            q_sb, k_sb, v_sb = q_pair[bi], k_pair[bi], v_pair[bi]
            vt3 = vt_pair[bi].rearrange("p (t c) -> p t c", c=130)
            # ---------------- GroupNorm stats -> (sc, bi) per c-tile --------
            x_sb = xp.tile([128, CT, N], BF16, tag="x", bufs=2)
            # two halves on the gpsimd queue: decoupled from the out DMAs
            # (sync queue) and lets ct 0/1 stats start during the second half
            nc.gpsimd.dma_start(x_sb[:, 0:2, :], xbf[:, 0:2 * N])
            nc.gpsimd.dma_start(x_sb[:, 2:4, :], xbf[:, 2 * N:4 * N])
            gs_ps = ps.tile([32, 2], F32, tag="qk", bufs=2, name="gs",
                            padded_shape=[32, 1024])
            for ct in range(CT):
                stats = work.tile([128, 5, 6], F32, tag=f"st{ct}")
                for i in range(4):
                    nc.vector.bn_stats(stats[:, i, :],
                                       x_sb[:, ct, i * 512:(i + 1) * 512])
                nc.vector.bn_stats(stats[:, 4, :], x_sb[:, ct, 2048:2304])
                mv = work.tile([128, 2], F32, tag=f"mv{ct}")
                nc.vector.bn_aggr(mv, stats)
                m1m2 = work.tile([128, 2], F32, tag=f"mm{ct}")
                nc.vector.tensor_copy(m1m2[:, 0:1], mv[:, 0:1])
                nc.vector.tensor_scalar(m1m2[:, 1:2], mv[:, 0:1], mv[:, 0:1],
                                        mv[:, 1:2], op0=ALU.mult, op1=ALU.add)
                nc.tensor.matmul(gs_ps, ind_sb[:, ct * 32:(ct + 1) * 32], m1m2,
                                 start=(ct == 0), stop=(ct == CT - 1))

            gs_sb = work.tile([32, 2], F32)
            nc.vector.tensor_copy(gs_sb, gs_ps)
            mu2 = work.tile([32, 1], F32)
            nc.vector.tensor_tensor(mu2, gs_sb[:, 0:1], gs_sb[:, 0:1],
                                    op=ALU.mult)
            gvar = work.tile([32, 1], F32)
            nc.vector.tensor_tensor(gvar, gs_sb[:, 1:2], mu2, op=ALU.subtract)
            # rstd = exp(-0.5 * ln(var + eps))  (keeps ACT on one table set)
            lnv = work.tile([32, 1], F32)
            nc.scalar.activation(lnv, gvar, AF.Ln, bias=eps_t)
            grs = work.tile([32, 2], F32)
            nc.vector.tensor_copy(grs[:, 0:1], gs_sb[:, 0:1])
            nc.scalar.activation(grs[:, 1:2], lnv, AF.Exp, scale=-0.5)

            # per-channel (sc, bi); fold sc into the qkv weights
            weff = wef.tile([128, 1536], MMDT, tag="w")
            wq_e, wk_e, wv_e = (weff[:, 0:512], weff[:, 512:1024],
                                weff[:, 1024:1536])
            bis = []
            for ct in range(CT):
                chs_ps = ps.tile([128, 2], F32, tag="qk", bufs=2,
                                 padded_shape=[128, 1024], name=f"chs{ct}")
                nc.tensor.matmul(chs_ps, indT_sb[:, ct * 128:(ct + 1) * 128],
                                 grs, start=True, stop=True)
                chs = work.tile([128, 2], F32, tag=f"ch{ct}")
                nc.vector.tensor_copy(chs, chs_ps)
                sc = work.tile([128, 1], F32, tag=f"sc{ct}")
                nc.vector.tensor_tensor(sc, chs[:, 1:2], gnsc_sb[:, ct:ct + 1],
                                        op=ALU.mult)
                bi = work.tile([128, 1], F32, tag=f"bif{ct}")
                nc.vector.tensor_tensor(bi, chs[:, 0:1], sc, op=ALU.mult)
                bi_b = work.tile([128, 1], BF16, tag=f"bi{ct}")
                nc.vector.tensor_tensor(bi_b, gnbi_sb[:, ct:ct + 1], bi,
                                        op=ALU.subtract)
                bis.append(bi_b)
                o = ct * 128
                nc.vector.tensor_scalar(wq_e[:, o:o + 128], wq_sb[:, o:o + 128],
                                        sc, None, op0=ALU.mult)
                nc.vector.tensor_scalar(wk_e[:, o:o + 128], wk_sb[:, o:o + 128],
                                        sc, None, op0=ALU.mult)
                nc.vector.tensor_scalar(wv_e[:, o:o + 128], wv_sb[:, o:o + 128],
                                        sc, None, op0=ALU.mult)

            # runtime bias: b_eff = W @ bi (+ conv bias for q/k)
            btot = wef.tile([128, 3], F32, tag="b")
            for wi, wsb in enumerate((wq_sb, wk_sb, wv_sb)):
                be_ps = ps.tile([128, 1], F32, tag="qk", bufs=2,
                                padded_shape=[128, 1024], name=f"be{wi}")
                for ct in range(CT):
                    nc.tensor.matmul(be_ps, wsb[:, ct * 128:(ct + 1) * 128],
                                     bis[ct], start=(ct == 0),
                                     stop=(ct == CT - 1))
                if wi == 0:
                    nc.vector.tensor_scalar(btot[:, 0:1], be_ps, bq_sb, None,
                                            op0=ALU.add)
                elif wi == 1:
                    nc.vector.tensor_scalar(btot[:, 1:2], be_ps, bk_sb, None,
                                            op0=ALU.add)
                else:
                    nc.vector.tensor_copy(btot[:, 2:3], be_ps)

            # ---------------- q/k/v chunks --------------------------------
            def kq_chunk(which, ci):
                c0, cw = CHUNKS[ci]
                w_e = wk_e if which == "k" else wq_e
                dst = k_sb if which == "k" else q_sb
                bcol = 1 if which == "k" else 0
                p = ps.tile([128, cw], F32, tag="qk", bufs=2,
                            padded_shape=[128, 1024], name=f"{which}{ci}")
                for ct in range(CT):
                    nc.tensor.matmul(p, w_e[:, ct * 128:(ct + 1) * 128],
                                     x_sb[:, ct, c0:c0 + cw],
                                     start=(ct == 0), stop=(ct == CT - 1))
                nc.vector.tensor_scalar(dst[:, c0:c0 + cw], p,
                                        btot[:, bcol:bcol + 1], None,
                                        op0=ALU.add)

            def v_chunk(ci):
                c0, cw = CHUNKS[ci]
                v_ps = ps.tile([128, cw], F32, tag="tr",
                               padded_shape=[128, 512], name=f"v{ci}")
                for ct in range(CT):
                    nc.tensor.matmul(v_ps, wv_e[:, ct * 128:(ct + 1) * 128],
                                     x_sb[:, ct, c0:c0 + cw],
                                     start=(ct == 0), stop=(ct == CT - 1))
                nc.vector.tensor_scalar(v_sb[:, c0:c0 + cw], v_ps,
                                        btot[:, 2:3], None, op0=ALU.add)
                for t in range(c0 // 128, (c0 + cw) // 128):
                    tr_ps = ps.tile([128, 128], MMDT, tag="tr", name=f"tr{t}")
                    nc.tensor.transpose(tr_ps, v_sb[:, t * 128:(t + 1) * 128],
                                        ident)
                    nc.vector.tensor_copy(vt3[:, t, 0:64], tr_ps[:, 0:64])
                    nc.vector.tensor_copy(vt3[:, t, 65:129], tr_ps[:, 64:128])

            # ---------------- attention helpers ---------------------------
            def qk_exp(ci, t):
                c0, cw = CHUNKS[ci]
                # head B's QK at column offset 512 so the two concurrent
                # matmuls never share a PSUM bank
                qk_ps = ps.tile([128, 1024], F32, tag="qk", bufs=2,
                                name=f"qk{ci}_{t}")
                e_sb = att.tile([128, 1024], MMDT, tag="e", bufs=PRO + 2,
                                name=f"e{ci}_{t}")
                nc.tensor.matmul(qk_ps[:, 0:cw],
                                 k_sb[0:64, t * 128:(t + 1) * 128],
                                 q_sb[0:64, c0:c0 + cw], start=True, stop=True)
                nc.tensor.matmul(qk_ps[:, 512:512 + cw],
                                 k_sb[64:128, t * 128:(t + 1) * 128],
                                 q_sb[64:128, c0:c0 + cw], start=True,
                                 stop=True)
                if cw == 512:
                    nc.scalar.activation(e_sb, qk_ps, AF.Exp, scale=SCALE)
                else:
                    # single strided exp over both heads' blocks
                    src = qk_ps.rearrange("p (b c) -> p b c", c=512)[:, :, 0:cw]
                    dst = e_sb.rearrange("p (b c) -> p b c", c=cw)[:, 0:2, :]
                    nc.scalar.activation(dst, src, AF.Exp, scale=SCALE)
                return e_sb

            def av(u, ci, e_sb, t):
                c0, cw = CHUNKS[ci]
                st, sp = (t == 0), (t == NT - 1)
                eB = e_sb[:, 512:512 + cw] if cw == 512 else e_sb[:, cw:2 * cw]
                nc.tensor.matmul(u[:, 0:cw], vt3[:, t, 0:65],
                                 e_sb[:, 0:cw], start=st, stop=sp)
                nc.tensor.matmul(u[:, 512:512 + cw], vt3[:, t, 65:130],
                                 eB, start=st, stop=sp)

            def norm(u, ci):
                c0, cw = CHUNKS[ci]
                dn = nrm.tile([1, 1024], F32, tag="dn", name=f"dn{ci}")
                rc = nrm.tile([1, 1024], F32, tag="rc", name=f"rc{ci}")
                bc = nrm.tile([64, 1024], F32, tag="bc", name=f"bc{ci}")
                a_t = nrm.tile([64, 1024], MMDT, tag="at", bufs=2,
                               name=f"at{ci}")
                if cw == 512:
                    nc.vector.tensor_copy(dn, u[64:65, :])
                    nc.vector.reciprocal(rc, dn)
                    nc.gpsimd.partition_broadcast(bc, rc, channels=64)
                    nc.vector.tensor_tensor(a_t, u[0:64, :], bc, op=ALU.mult)
                else:
                    u3 = u.rearrange("p (b c) -> p b c", c=512)[:, :, 0:cw]
                    dn2 = dn.rearrange("p (b c) -> p b c", c=cw)[:, 0:2, :]
                    nc.vector.tensor_copy(dn2, u3[64:65])
                    nc.vector.reciprocal(rc[:, 0:2 * cw], dn[:, 0:2 * cw])
                    nc.gpsimd.partition_broadcast(bc[:, 0:2 * cw],
                                                  rc[:, 0:2 * cw], channels=64)
                    at2 = a_t.rearrange("p (b c) -> p b c", c=cw)[:, 0:2, :]
                    nc.vector.tensor_tensor(at2, u3[0:64], bc.rearrange(
                        "p (b c) -> p b c", c=cw)[:, 0:2, :], op=ALU.mult)
                return a_t

            def proj(a_t, ci, tags=("pp", "tr")):
                # a_t layout: head A at cols 0:cw, head B at cw:2cw (tail)
                # or 512:512+cw (cw=512); psum tags alternate so the WAR on
                # the o-copy is distance-2
                c0, cw = CHUNKS[ci]
                aB0 = 512 if cw == 512 else cw
                for mt in range(4):
                    tg = tags[mt % 2]
                    p_ps = ps.tile([128, cw], F32, tag=tg,
                                   bufs=2 if tg == "qk" else None,
                                   padded_shape=[128, 1024]
                                   if tg == "qk" else [128, 512],
                                   name=f"pp{ci}_{mt}")
                    nc.tensor.matmul(p_ps, wpa_sb[:, mt * 128:(mt + 1) * 128],
                                     a_t[:, 0:cw], start=True, stop=False)
                    nc.tensor.matmul(p_ps, wpb_sb[:, mt * 128:(mt + 1) * 128],
                                     a_t[:, aB0:aB0 + cw], start=False,
                                     stop=True)
                    o_sb = att.tile([128, cw], F32, tag="o", bufs=4,
                                    padded_shape=[128, 512], name=f"o{ci}_{mt}")
                    nc.vector.tensor_copy(o_sb, p_ps)
                    nc.sync.dma_start(out[mt * 128:(mt + 1) * 128,
                                          c0:c0 + cw], o_sb)

            # ---------------- schedule ------------------------------------
            # k/q chunk 0 then the first prologue QKs immediately, so ACT
            # reaches the first exp as early as possible after an iteration
            # boundary; v/k/q chunks stream in behind.
            kq_chunk("k", 0)
            kq_chunk("q", 0)

            u0 = ps.tile([65, 1024], F32, tag="u", name="u0")
            # prologue interleaved with k/v chunk prep: a QK at tile t needs
            # k chunk t//4 already emitted (PRO <= 9 keeps this satisfiable)
            es = {}
            for t in range(PRO):
                es[t] = qk_exp(0, t)
                if t == 3:
                    kq_chunk("k", 1)
                elif t == 5:
                    v_chunk(0)
                elif t == 7:
                    v_chunk(1)
                    kq_chunk("k", 2)
            prev = (u0, 0)
            # NOTE: a k/v chunk must be emitted BEFORE any qk_exp/av that
            # reads its tiles — qk_exp(t+PRO) at step t reads k tile t+PRO
            for t in range(NT):
                av(u0, 0, es.pop(t), t)
                if t + PRO < NT:
                    es[t + PRO] = qk_exp(0, t + PRO)
                if t == 0:
                    v_chunk(2)
                elif t == 2:
                    kq_chunk("k", 3)
                elif t == 4:
                    v_chunk(3)
                elif t == 6:
                    kq_chunk("k", 4)
                elif t == 8:
                    v_chunk(4)
                elif t == 12:
                    kq_chunk("q", 1)

            # chunks 1..4, software-pipelined; norm(prev) in the prologue
            # shadow, proj(prev) a few steady steps in
            for ci in range(1, len(CHUNKS)):
                u = ps.tile([65, 1024], F32, tag="u", name=f"u{ci}")
                es = {t: qk_exp(ci, t) for t in range(PRO)}
                pu, pci = prev
                pa_t = norm(pu, pci)
                for t in range(NT):
                    av(u, ci, es.pop(t), t)
                    if t + PRO < NT:
                        es[t + PRO] = qk_exp(ci, t + PRO)
                    if t == 1 and ci + 1 < len(CHUNKS):
                        kq_chunk("q", ci + 1)
                    elif t == 3:
                        proj(pa_t, pci)
                prev = (u, ci)

            pu, pci = prev
            pa_t = norm(pu, pci)
            proj(pa_t, pci, tags=("qk", "pp"))

        with nc.allow_low_precision(reason="bf16 compute pipeline by design"), \
                (tc.For_i(0, repeat // unroll, 1) if repeat
                 else nullcontext()):
            for u_i in range(unroll):
                body(u_i % 2)

    nc.compile()
    return nc


def _prep_core_inputs(core, xf, gn_w, gn_b, qkv_w, qkv_b, proj_w):
    """Per-core input dict. core -> (batch, head pair)."""
    b = core // 4
    hA, hB = 2 * (core % 4), 2 * (core % 4) + 1
    heads = [hA] * 64 + [hB] * 64
    dims = list(range(64)) + list(range(64))
    q_rows = np.array([h * 192 + d * 3 + 0 for h, d in zip(heads, dims)])
    k_rows = q_rows + 1
    v_rows = q_rows + 2

    # fpk: [wq(512) | wk(512) | wv(512) | ident(128) | ones(37)], c-tile major
    def wtiles(rows):
        m = qkv_w[rows, :].T.reshape(CT, 128, 128)        # [ct][c_in, out]
        return np.concatenate([m[ct] for ct in range(CT)], axis=1)

    fpk_m = np.concatenate(
        [wtiles(q_rows), wtiles(k_rows), wtiles(v_rows),
         np.eye(128, dtype=np.float32), np.ones((128, 37), np.float32)],
        axis=1)

    wp_m = np.concatenate([proj_w[:, hA * 64:(hA + 1) * 64].T,
                           proj_w[:, hB * 64:(hB + 1) * 64].T], axis=1)

    ch = np.arange(C)
    grp = ch // 16
    ind_m = np.zeros((C, 32), np.float32)
    ind_m[ch, grp] = 1.0 / 16.0

    ind_cols = np.concatenate(
        [ind_m.reshape(CT, 128, 32)[ct] for ct in range(CT)], axis=1)
    indT_m = np.zeros((32, C), np.float32)
    indT_m[grp, ch] = 1.0
    indT_cols = np.concatenate(
        [indT_m.reshape(32, CT, 128)[:, ct, :] for ct in range(CT)], axis=1)

    cpk_m = np.concatenate(
        [ind_cols,
         gn_w.reshape(CT, 128).T, gn_b.reshape(CT, 128).T,
         qkv_b[q_rows].reshape(128, 1), qkv_b[k_rows].reshape(128, 1)], axis=1)

    # x c-tile major [128, 4*2304] bf16
    x_ct = xf[b].reshape(CT, 128, N)
    x_m = np.concatenate([x_ct[ct] for ct in range(CT)], axis=1)

    mmnp = ml_dtypes.bfloat16
    return {
        "xbf": np.ascontiguousarray(x_m).astype(mmnp),
        "fpk": np.ascontiguousarray(fpk_m).astype(mmnp),
        "wp": np.ascontiguousarray(wp_m).astype(mmnp),
        "cpk": np.ascontiguousarray(cpk_m, np.float32),
        "indT": np.ascontiguousarray(indT_cols, np.float32),
    }


last_result = None  # BassKernelResults of the most recent run (for profiling)


def kernel(x, gn_w, gn_b, qkv_w, qkv_b, proj_w, proj_b, *, trace=False):
    x = np.asarray(x, np.float32)
    gn_w = np.asarray(gn_w, np.float32)
    gn_b = np.asarray(gn_b, np.float32)
    qkv_w = np.asarray(qkv_w, np.float32)
    qkv_b = np.asarray(qkv_b, np.float32)
    proj_w = np.asarray(proj_w, np.float32)
    proj_b = np.asarray(proj_b, np.float32)

    if "nc" not in _CACHE:
        _CACHE["nc"] = _build()
    nc = _CACHE["nc"]

    xf = x.reshape(B, C, N)
    in_maps = [_prep_core_inputs(c, xf, gn_w, gn_b, qkv_w, qkv_b, proj_w)
               for c in range(NCORES)]

    res = bass_utils.run_bass_kernel_spmd(nc, in_maps,
                                          core_ids=list(range(NCORES)),
                                          trace=trace)
    global last_result
    last_result = res

    # v-bias folds to a constant per-channel vector through softmax + proj
    bv = qkv_b[np.array([h * 192 + d * 3 + 2 for h in range(HEADS)
                         for d in range(D)])]
    cv = proj_w @ bv + proj_b                                  # [C]

    outp = np.zeros((B, C, N), np.float32)
    for core in range(NCORES):
        outp[core // 4] += res.results[core]["out"]
    outp += cv[None, :, None]
    outp += xf
    return outp.reshape(B, C, H, W)
